# revision 2
# baseline (speedup 1.0000x reference)
"""CommNet MoE-routing kernel for 8 Trainium2 NeuronCores.

Strategy: expert-parallel over the 32 models (4 models per core, greedy
balanced by routed-example count).  The host groups examples by model id,
pads each group to a uniform width P, and packs per-core inputs:

  - wbig [128, CW]: every weight tile [128 x cols] laid out in the exact
    order the PE consumes it (phase A: Wc/Wr; then Wa|Wb merged; then Wo
    ob-major).  Weights are used exactly once -> pure streaming; total
    weight traffic across the fleet is minimal (each bank read once).
  - xin  [128, XCOLS]: activations transposed to [H, n] layout (H on
    partitions), plus per-model biases, identity, etc.

On device (identical SPMD program, data differs per core):
  phase A: comm agent-sum (DVE) then per-group matmuls accumulate
           Wc.T@comm_sum + Wr.T@prev_hid into PSUM; batched epilogue
           adds enc + (bc+br) and applies tanh -> hidstate^T.
  phase B: merged [Wa|Wb] matmul -> [17, N] -> PE transpose -> softmax
           (exp/sum/recip) over the 16 actions, passthrough baseline;
           Wo streamed ob-major, bias+1/7 scale fused in DVE tensor_scalar.

Outputs come back transposed; the host scatters rows back to the original
example order.
"""

import os
import sys
import types

import numpy as np

# The image's antenv lacks axon_hooks; shim it so trace=True works.
if "antenv.axon_hooks" not in sys.modules:
    try:
        from trn_agent_boot.trn_boot import _ntff_profile_via_ctypes

        _hook = _ntff_profile_via_ctypes("/opt/axon/libaxon_pjrt.so")
        _m = types.ModuleType("antenv.axon_hooks")
        _m.get_axon_ntff_profile_hook = lambda: _hook
        _m.set_axon_ntff_profile_hook = lambda h: None
        sys.modules["antenv.axon_hooks"] = _m
    except Exception:
        pass

import concourse.bass as bass
import concourse.mybir as mybir
from concourse.tile import TileContext
from concourse.bass_utils import run_bass_kernel_spmd

F32 = mybir.dt.float32

B, M, NAG, H, NACT, O = 512, 32, 8, 256, 16, 2048
NCORES, MPC = 8, 4  # cores, models per core
AB = NACT + 1  # merged action|baseline output width

# Set by test.py to capture profiling results of the last run.
LAST_RESULTS = None
TRACE = bool(os.environ.get("BASS_TRACE"))

_PROG_CACHE = {}


def _split_sync_waits(nc, max_waits=1):
    """Walrus codegen rejects instructions with too many sem waits; move
    extras onto EventSemaphore carriers inserted just before."""
    for bb in nc.main_func.blocks:
        new_insts, changed = [], False
        for ins in bb.instructions:
            si = ins.sync_info
            if si is not None and len(si.on_wait) > max_waits:
                waits = list(si.on_wait)
                movable = [w for w in waits if w.wait_reg is None]
                pinned = [w for w in waits if w.wait_reg is not None]
                keep_n = max(max_waits - len(pinned), 0)
                cut = len(movable) - keep_n
                extra, keep = movable[:cut], pinned + movable[cut:]
                for j, w in enumerate(extra):
                    ev = mybir.InstEventSemaphore(
                        name=f"{ins.name}_sw{j}", ins=[], outs=[]
                    )
                    ev.engine = ins.engine
                    ev.sync_info = mybir.SyncInfo(on_wait=[w], on_update=[])
                    new_insts.append(ev)
                ins.sync_info = mybir.SyncInfo(on_wait=keep, on_update=list(si.on_update))
                changed = True
            new_insts.append(ins)
        if changed:
            bb.instructions = new_insts


def _build_program(P):
    """One SPMD program, parameterized only by the padded group width P."""
    N = MPC * P
    assert N <= 512, f"padded batch per core {N} exceeds one PSUM bank"
    # xin column layout (host packs to match)
    c_comm = 0              # 16N  (a, i, n)
    c_hidp = 16 * N         # 2N   (i, n)
    c_enc = 18 * N          # 2N   (i, n)
    c_bh = 20 * N           # 8    (i*4+g)   bc+br per model, per H-block
    c_bo = 20 * N + 8       # 64   (g*16+ob) bo per model per O-block
    c_id = 20 * N + 72      # 128  identity
    c_ab = 20 * N + 200     # 4    [ba; bb] per model (17 partitions used)
    XCOLS = 20 * N + 204
    # wbig column layout
    c_wab = MPC * 1024      # 4096
    c_wo = c_wab + MPC * 2 * AB  # 4232
    CW = c_wo + 16 * 1024   # 20616

    nc = bass.Bass(target_bir_lowering=False)
    xin_d = nc.dram_tensor("xin", [128, XCOLS], F32, kind="ExternalInput")
    wbig_d = nc.dram_tensor("wbig", [128, CW], F32, kind="ExternalInput")
    act_d = nc.dram_tensor("oact", [N, AB], F32, kind="ExternalOutput")
    hid_d = nc.dram_tensor("ohid", [2, 128, N], F32, kind="ExternalOutput")
    com_d = nc.dram_tensor("ocom", [16, 128, N], F32, kind="ExternalOutput")

    gsl = lambda g: slice(g * P, (g + 1) * P)

    with TileContext(nc) as tc:
        with (
            tc.tile_pool(name="persist", bufs=1) as pp,
            tc.tile_pool(name="work", bufs=2) as wp,
            tc.tile_pool(name="co", bufs=3) as cop,
            tc.tile_pool(name="ps1", bufs=1, space="PSUM") as ps1,
            tc.tile_pool(name="ptr", bufs=2, space="PSUM") as ptrp,
            tc.tile_pool(name="po", bufs=3, space="PSUM") as pop,
        ):
            # Warm the ACT function table early (tanh/exp share a set).
            dummy = pp.tile([1, 1], F32, tag="dummy", name="dummy")
            nc.vector.memset(dummy[:, :], 0.0)
            nc.scalar.activation(dummy[:, :], dummy[:, :], mybir.ActivationFunctionType.Tanh)

            xin = pp.tile([128, XCOLS], F32, tag="xin", name="xin_sb")
            nc.sync.dma_start(out=xin[:, :], in_=xin_d[:, :])

            wA = []
            for g in range(MPC):
                t = pp.tile([128, 1024], F32, tag=f"wA{g}", name=f"wA{g}")
                nc.sync.dma_start(out=t[:, :], in_=wbig_d[:, g * 1024:(g + 1) * 1024])
                wA.append(t)
            wab = pp.tile([128, MPC * 2 * AB], F32, tag="wab", name="wab_sb")
            nc.sync.dma_start(out=wab[:, :], in_=wbig_d[:, c_wab:c_wo])
            wO = []
            for ob in range(16):
                t = pp.tile([128, 1024], F32, tag=f"wO{ob}", name=f"wOsb{ob}")
                nc.sync.dma_start(
                    out=t[:, :], in_=wbig_d[:, c_wo + ob * 1024:c_wo + (ob + 1) * 1024]
                )
                wO.append(t)

            # comm agent-sum: cs[i] = sum_a comm[a, i]
            def comm_sl(a, i):
                c0 = c_comm + (a * 2 + i) * N
                return xin[:, c0:c0 + N]

            cs = []
            for i in range(2):
                t = pp.tile([128, N], F32, tag=f"cs{i}", name=f"cs{i}")
                nc.vector.tensor_add(out=t[:, :], in0=comm_sl(0, i), in1=comm_sl(1, i))
                for a in range(2, NAG):
                    nc.vector.tensor_add(out=t[:, :], in0=t[:, :], in1=comm_sl(a, i))
                cs.append(t)

            def hp(i):
                return xin[:, c_hidp + i * N:c_hidp + (i + 1) * N]

            # phase A matmuls: psum1[ob][:, g] += sum_j Wj.T @ x_j
            psum1 = [ps1.tile([128, N], F32, tag=f"p1{ob}", name=f"p1{ob}") for ob in range(2)]
            rhs_for_j = [cs[0], cs[1], hp(0), hp(1)]
            for g in range(MPC):
                for ob in range(2):
                    for j in range(4):
                        wt = wA[g][:, (ob * 4 + j) * 128:(ob * 4 + j + 1) * 128]
                        nc.tensor.matmul(
                            psum1[ob][:, gsl(g)], wt, rhs_for_j[j][:, gsl(g)],
                            start=(j == 0), stop=(j == 3),
                        )

            # phase A epilogue: hid = tanh(psum + enc + (bc+br))
            hid = []
            for ob in range(2):
                pre = wp.tile([128, N], F32, tag="pre", name="pre")
                nc.vector.tensor_add(
                    out=pre[:, :], in0=psum1[ob][:, :],
                    in1=xin[:, c_enc + ob * N:c_enc + (ob + 1) * N],
                )
                for g in range(MPC):
                    nc.vector.tensor_scalar_add(
                        pre[:, gsl(g)], pre[:, gsl(g)],
                        xin[:, c_bh + ob * 4 + g:c_bh + ob * 4 + g + 1],
                    )
                h = pp.tile([128, N], F32, tag=f"hid{ob}", name=f"hid{ob}")
                nc.scalar.activation(h[:, :], pre[:, :], mybir.ActivationFunctionType.Tanh)
                nc.sync.dma_start(out=hid_d[ob, :, :], in_=h[:, :])
                hid.append(h)

            # action|baseline: [17, N] = [Wa|Wb].T @ hid + [ba; bb]
            pab = ps1.tile([AB, N], F32, tag="pab", name="pab")
            for g in range(MPC):
                for i in range(2):
                    wt = wab[:, (g * 2 + i) * AB:(g * 2 + i + 1) * AB]
                    nc.tensor.matmul(
                        pab[:, gsl(g)], wt, hid[i][:, gsl(g)],
                        start=(i == 0), stop=(i == 1),
                    )
            absb = pp.tile([AB, N], F32, tag="absb", name="absb")
            for g in range(MPC):
                nc.vector.tensor_scalar_add(
                    absb[:, gsl(g)], pab[:, gsl(g)], xin[:AB, c_ab + g:c_ab + g + 1]
                )
            # transpose chunks of <=128 examples; softmax rows (free dim = 16)
            for ch0 in range(0, N, 128):
                cw = min(128, N - ch0)
                ptr = ptrp.tile([128, AB], F32, tag="ptr", name="ptr")
                nc.tensor.transpose(
                    ptr[:cw, :], absb[:, ch0:ch0 + cw], xin[:AB, c_id:c_id + AB]
                )
                ex = wp.tile([128, NACT], F32, tag="ex", name="ex")
                nc.scalar.activation(
                    ex[:cw, :], ptr[:cw, :NACT], mybir.ActivationFunctionType.Exp
                )
                red = wp.tile([128, 1], F32, tag="red", name="red")
                nc.vector.reduce_sum(out=red[:cw, :], in_=ex[:cw, :], axis=mybir.AxisListType.X)
                rec = wp.tile([128, 1], F32, tag="rec", name="rec")
                nc.vector.reciprocal(rec[:cw, :], red[:cw, :])
                aout = wp.tile([128, AB], F32, tag="aout", name="aout")
                nc.vector.tensor_scalar_mul(aout[:cw, :NACT], ex[:cw, :], rec[:cw, :])
                nc.vector.tensor_copy(out=aout[:cw, NACT:AB], in_=ptr[:cw, NACT:AB])
                nc.sync.dma_start(out=act_d[ch0:ch0 + cw, :], in_=aout[:cw, :])

            # comm_out: ob-major streaming; co = (psum + bo) / (NAG-1)
            for ob in range(16):
                po = pop.tile([128, N], F32, tag="po", name="po")
                for g in range(MPC):
                    for i in range(2):
                        wt = wO[ob][:, (g * 2 + i) * 128:(g * 2 + i + 1) * 128]
                        nc.tensor.matmul(
                            po[:, gsl(g)], wt, hid[i][:, gsl(g)],
                            start=(i == 0), stop=(i == 1),
                        )
                co = cop.tile([128, N], F32, tag="co", name="co")
                for g in range(MPC):
                    nc.vector.tensor_scalar(
                        co[:, gsl(g)], po[:, gsl(g)],
                        xin[:, c_bo + g * 16 + ob:c_bo + g * 16 + ob + 1],
                        1.0 / (NAG - 1),
                        op0=mybir.AluOpType.add, op1=mybir.AluOpType.mult,
                    )
                nc.sync.dma_start(out=com_d[ob, :, :], in_=co[:, :])

    _split_sync_waits(nc)
    layout = dict(N=N, c_comm=c_comm, c_hidp=c_hidp, c_enc=c_enc, c_bh=c_bh,
                  c_bo=c_bo, c_id=c_id, c_ab=c_ab, XCOLS=XCOLS, c_wab=c_wab,
                  c_wo=c_wo, CW=CW)
    return nc, layout


def _pack_core(L, P, models, groups, comm_in, prev_hid, encg,
               Wc, bc, Wr, br, Wa, ba, Wb, bb, Wo, bo):
    """Build (xin, wbig) for one core."""
    N = L["N"]
    sel = np.zeros(N, np.int64)
    nvalid = []
    for g, m in enumerate(models):
        ii = groups[m]
        sel[g * P:g * P + len(ii)] = ii
        nvalid.append(len(ii))
    valid = np.zeros(N, bool)
    for g in range(MPC):
        valid[g * P:g * P + nvalid[g]] = True

    xin = np.zeros((128, L["XCOLS"]), np.float32)
    cg = comm_in[sel]          # [N, 8, 256]
    cg[~valid] = 0
    # [N,8,256] -> [128, (a,i,n)]
    xin[:, L["c_comm"]:L["c_comm"] + 16 * N] = (
        cg.reshape(N, NAG, 2, 128).transpose(3, 1, 2, 0).reshape(128, 16 * N)
    )
    hg = prev_hid[sel]
    hg[~valid] = 0
    xin[:, L["c_hidp"]:L["c_hidp"] + 2 * N] = (
        hg.reshape(N, 2, 128).transpose(2, 1, 0).reshape(128, 2 * N)
    )
    eg = encg[sel]
    eg[~valid] = 0
    xin[:, L["c_enc"]:L["c_enc"] + 2 * N] = (
        eg.reshape(N, 2, 128).transpose(2, 1, 0).reshape(128, 2 * N)
    )
    bh = bc[models] + br[models]  # [4, 256]
    xin[:, L["c_bh"]:L["c_bh"] + 8] = bh.reshape(MPC, 2, 128).transpose(2, 1, 0).reshape(128, 8)
    xin[:, L["c_bo"]:L["c_bo"] + 64] = (
        bo[models].reshape(MPC, 16, 128).transpose(2, 0, 1).reshape(128, 64)
    )
    xin[:, L["c_id"]:L["c_id"] + 128] = np.eye(128, dtype=np.float32)
    xin[:NACT, L["c_ab"]:L["c_ab"] + MPC] = ba[models].T
    xin[NACT, L["c_ab"]:L["c_ab"] + MPC] = bb[models, 0]

    wbig = np.empty((128, L["CW"]), np.float32)
    for g, m in enumerate(models):
        blk = np.empty((2, 4, 128, 128), np.float32)  # [ob, j, p, col]
        for ob in range(2):
            for i in range(2):
                blk[ob, i] = Wc[m, i * 128:(i + 1) * 128, ob * 128:(ob + 1) * 128]
                blk[ob, 2 + i] = Wr[m, i * 128:(i + 1) * 128, ob * 128:(ob + 1) * 128]
        wbig[:, g * 1024:(g + 1) * 1024] = blk.transpose(2, 0, 1, 3).reshape(128, 1024)
        for i in range(2):
            c0 = L["c_wab"] + (g * 2 + i) * AB
            wbig[:, c0:c0 + NACT] = Wa[m, i * 128:(i + 1) * 128, :]
            wbig[:, c0 + NACT:c0 + AB] = Wb[m, i * 128:(i + 1) * 128, :]
    for ob in range(16):
        c0 = L["c_wo"] + ob * 1024
        for g, m in enumerate(models):
            for i in range(2):
                wbig[:, c0 + (g * 2 + i) * 128:c0 + (g * 2 + i + 1) * 128] = (
                    Wo[m, i * 128:(i + 1) * 128, ob * 128:(ob + 1) * 128]
                )
    return xin, wbig


def kernel(comm_in, inp, prev_hid, prev_cell, model_ids,
           Wc, bc, Wr, br, Wa, ba, Wb, bb, Wo, bo, lut, enc_bias):
    global LAST_RESULTS
    f = lambda x: np.ascontiguousarray(np.asarray(x), dtype=np.float32)
    comm_in, prev_hid = f(comm_in), f(prev_hid)
    Wc, bc, Wr, br = f(Wc), f(bc), f(Wr), f(br)
    Wa, ba, Wb, bb, Wo, bo = f(Wa), f(ba), f(Wb), f(bb), f(Wo), f(bo)
    lut, enc_bias = f(lut), f(enc_bias)
    ids = np.asarray(model_ids).astype(np.int64)
    inp_i = np.asarray(inp).astype(np.int64)

    # Encoder (embedding gather + bias) — index logistics done host-side.
    encg = lut[np.clip(inp_i[:, 0], 0, lut.shape[0] - 1)] + enc_bias  # [B, 256]

    # Greedy balanced assignment of models to cores.
    counts = np.bincount(ids, minlength=M)
    groups = [np.where(ids == m)[0] for m in range(M)]
    order = np.argsort(-counts, kind="stable")
    loads, core_models = [0] * NCORES, [[] for _ in range(NCORES)]
    for m in order:
        cands = [c for c in range(NCORES) if len(core_models[c]) < MPC]
        c = min(cands, key=lambda c: loads[c])
        core_models[c].append(int(m))
        loads[c] += int(counts[m])
    P = max(4, int(-(-int(counts.max()) // 4)) * 4)

    key = P
    if key not in _PROG_CACHE:
        _PROG_CACHE[key] = _build_program(P)
    nc, L = _PROG_CACHE[key]
    N = L["N"]

    in_maps = []
    for c in range(NCORES):
        xin, wbig = _pack_core(L, P, core_models[c], groups, comm_in, prev_hid,
                               encg, Wc, bc, Wr, br, Wa, ba, Wb, bb, Wo, bo)
        in_maps.append({"xin": xin, "wbig": wbig})

    res = run_bass_kernel_spmd(nc, in_maps, core_ids=list(range(NCORES)), trace=TRACE)
    LAST_RESULTS = res

    action = np.zeros((B, NACT), np.float32)
    baseline = np.zeros((B, 1), np.float32)
    hidstate = np.zeros((B, H), np.float32)
    comm_out = np.zeros((B, O), np.float32)
    for c in range(NCORES):
        oact = res.results[c]["oact"]            # [N, 17]
        ohid = res.results[c]["ohid"].reshape(H, N)
        ocom = res.results[c]["ocom"].reshape(O, N)
        for g, m in enumerate(core_models[c]):
            ii = groups[m]
            n = len(ii)
            if n == 0:
                continue
            r = slice(g * P, g * P + n)
            action[ii] = oact[r, :NACT]
            baseline[ii, 0] = oact[r, NACT]
            hidstate[ii] = ohid[:, r].T
            comm_out[ii] = ocom[:, r].T
    return action, baseline, hidstate, comm_out


# revision 5
# speedup vs baseline: 1.7212x; 1.7212x over previous
"""CommNet MoE-routing kernel for 8 Trainium2 NeuronCores.

Strategy: expert-parallel over the 32 models (4 models per core, greedy
balanced by routed-example count).  The host groups examples by model id,
pads each group to a uniform width P, and packs per-core inputs:

  - wbig [128, CW]: every weight tile [128 x cols] laid out in the exact
    order the PE consumes it (phase A: Wc/Wr; then Wa|Wb merged; then Wo
    ob-major).  Weights are used exactly once -> pure streaming; total
    weight traffic across the fleet is minimal (each bank read once).
  - xin  [128, XCOLS]: activations transposed to [H, n] layout (H on
    partitions), plus per-model biases, identity, etc.

On device (identical SPMD program, data differs per core):
  phase A: comm agent-sum (DVE) then per-group matmuls accumulate
           Wc.T@comm_sum + Wr.T@prev_hid into PSUM; batched epilogue
           adds enc + (bc+br) and applies tanh -> hidstate^T.
  phase B: merged [Wa|Wb] matmul -> [17, N] -> PE transpose -> softmax
           (exp/sum/recip) over the 16 actions, passthrough baseline;
           Wo streamed ob-major, bias+1/7 scale fused in DVE tensor_scalar.

Outputs come back transposed; the host scatters rows back to the original
example order.
"""

import os
import sys
import types

import numpy as np

# The image's antenv lacks axon_hooks; shim it so trace=True works.
if "antenv.axon_hooks" not in sys.modules:
    try:
        from trn_agent_boot.trn_boot import _ntff_profile_via_ctypes

        _hook = _ntff_profile_via_ctypes("/opt/axon/libaxon_pjrt.so")
        _m = types.ModuleType("antenv.axon_hooks")
        _m.get_axon_ntff_profile_hook = lambda: _hook
        _m.set_axon_ntff_profile_hook = lambda h: None
        sys.modules["antenv.axon_hooks"] = _m
    except Exception:
        pass

import concourse.bass as bass
import concourse.mybir as mybir
from concourse.tile import TileContext
from concourse.bass_utils import run_bass_kernel_spmd

F32 = mybir.dt.float32
BF16 = mybir.dt.bfloat16
F32R = mybir.dt.float32r

B, M, NAG, H, NACT, O = 512, 32, 8, 256, 16, 2048
NCORES, MPC = 8, 4  # cores, models per core
AB = NACT + 1  # merged action|baseline output width

# Set by test.py to capture profiling results of the last run.
LAST_RESULTS = None
TRACE = bool(os.environ.get("BASS_TRACE"))

_PROG_CACHE = {}


def _split_sync_waits(nc, max_waits=1):
    """Walrus codegen rejects instructions with too many sem waits; move
    extras onto EventSemaphore carriers inserted just before."""
    for bb in nc.main_func.blocks:
        new_insts, changed = [], False
        for ins in bb.instructions:
            si = ins.sync_info
            if si is not None and len(si.on_wait) > max_waits:
                waits = list(si.on_wait)
                movable = [w for w in waits if w.wait_reg is None]
                pinned = [w for w in waits if w.wait_reg is not None]
                keep_n = max(max_waits - len(pinned), 0)
                cut = len(movable) - keep_n
                extra, keep = movable[:cut], pinned + movable[cut:]
                for j, w in enumerate(extra):
                    ev = mybir.InstEventSemaphore(
                        name=f"{ins.name}_sw{j}", ins=[], outs=[]
                    )
                    ev.engine = ins.engine
                    ev.sync_info = mybir.SyncInfo(on_wait=[w], on_update=[])
                    new_insts.append(ev)
                ins.sync_info = mybir.SyncInfo(on_wait=keep, on_update=list(si.on_update))
                changed = True
            new_insts.append(ins)
        if changed:
            bb.instructions = new_insts


def _build_program(P):
    """One SPMD program, parameterized only by the padded group width P."""
    N = MPC * P
    assert N <= 512, f"padded batch per core {N} exceeds one PSUM bank"
    assert P <= 128
    # xin column layout (host packs to match)
    c_comm = 0              # 16N  (a, i, n)
    c_hidp = 16 * N         # 2N   (i, n)
    c_enc = 18 * N          # 2N   (i, n)
    c_bh = 20 * N           # 8    (i*4+g)   bc+br per model, per H-block
    c_bo = 20 * N + 8       # 64   (g*16+ob) bo per model per O-block
    c_id = 20 * N + 72      # 128  identity
    c_ab = 20 * N + 200     # 4    [ba; bb] per model (17 partitions used)
    XCOLS = 20 * N + 204
    # wb (bf16) column layout: [wab | Wo ob-major]
    c_wo = MPC * 2 * AB     # 136
    CWB = c_wo + 16 * 1024

    nc = bass.Bass(target_bir_lowering=False)
    xin_d = nc.dram_tensor("xin", [128, XCOLS], F32, kind="ExternalInput")
    wa_d = nc.dram_tensor("wa", [128, MPC * 1024], F32, kind="ExternalInput")
    wb_d = nc.dram_tensor("wb", [128, CWB], BF16, kind="ExternalInput")
    act_d = nc.dram_tensor("oact", [N, AB], F32, kind="ExternalOutput")
    hid_d = nc.dram_tensor("ohid", [2, 128, N], F32, kind="ExternalOutput")
    com_d = nc.dram_tensor("ocom", [16, 128, N], F32, kind="ExternalOutput")

    gsl = lambda g: slice(g * P, (g + 1) * P)

    with TileContext(nc) as tc:
        with (
            tc.tile_pool(name="persist", bufs=1) as pp,
            tc.tile_pool(name="work", bufs=2) as wp,
            tc.tile_pool(name="co", bufs=3) as cop,
            tc.tile_pool(name="psA", bufs=2, space="PSUM") as psap,
            tc.tile_pool(name="ps1", bufs=1, space="PSUM") as ps1,
            tc.tile_pool(name="ptr", bufs=1, space="PSUM") as ptrp,
            tc.tile_pool(name="po", bufs=2, space="PSUM") as pop,
        ):
            # Warm the ACT function table early (tanh/exp share a set).
            dummy = pp.tile([1, 1], F32, tag="dummy", name="dummy")
            nc.vector.memset(dummy[:, :], 0.0)
            nc.scalar.activation(dummy[:, :], dummy[:, :], mybir.ActivationFunctionType.Tanh)

            xin = pp.tile([128, XCOLS], F32, tag="xin", name="xin_sb")
            nc.sync.dma_start(out=xin[:, :], in_=xin_d[:, :])

            wA = []
            for g in range(MPC):
                tf = pp.tile([128, 1024], F32, tag=f"wAf{g}", name=f"wAf{g}")
                nc.sync.dma_start(out=tf[:, :], in_=wa_d[:, g * 1024:(g + 1) * 1024])
                t = pp.tile([128, 1024], F32R, tag=f"wA{g}", name=f"wA{g}")
                nc.vector.tensor_copy(out=t[:, :], in_=tf[:, :])
                wA.append(t)
            wab = pp.tile([128, MPC * 2 * AB], BF16, tag="wab", name="wab_sb")
            nc.sync.dma_start(out=wab[:, :], in_=wb_d[:, 0:c_wo])
            wO = []
            for ob in range(16):
                t = pp.tile([128, 1024], BF16, tag=f"wO{ob}", name=f"wOsb{ob}")
                nc.sync.dma_start(
                    out=t[:, :], in_=wb_d[:, c_wo + ob * 1024:c_wo + (ob + 1) * 1024]
                )
                wO.append(t)

            # comm agent-sum: cs[i] = sum_a comm[a, i]
            def comm_sl(a, i):
                c0 = c_comm + (a * 2 + i) * N
                return xin[:, c0:c0 + N]

            cs = []
            for i in range(2):
                t = pp.tile([128, N], F32, tag=f"cs{i}", name=f"cs{i}")
                nc.vector.tensor_add(out=t[:, :], in0=comm_sl(0, i), in1=comm_sl(1, i))
                for a in range(2, NAG):
                    nc.vector.tensor_add(out=t[:, :], in0=t[:, :], in1=comm_sl(a, i))
                tr = pp.tile([128, N], F32R, tag=f"csr{i}", name=f"csr{i}")
                nc.vector.tensor_copy(out=tr[:, :], in_=t[:, :])
                cs.append(tr)

            hpr = []
            for i in range(2):
                t = pp.tile([128, N], F32R, tag=f"hpr{i}", name=f"hpr{i}")
                nc.vector.tensor_copy(
                    out=t[:, :], in_=xin[:, c_hidp + i * N:c_hidp + (i + 1) * N]
                )
                hpr.append(t)

            # phase A, flipped: examples stationary, weights moving (f32r,
            # N=256 -> full-rate).  psA_g[n, o] = cs.T@Wc + hp.T@Wr
            rhs_for_j = [cs[0], cs[1], hpr[0], hpr[1]]
            sbA = []
            for g in range(MPC):
                pA = psap.tile([128, 256], F32, tag="psA", name=f"psA{g}")
                for j in range(4):
                    nc.tensor.matmul(
                        pA[:P, :], rhs_for_j[j][:, gsl(g)],
                        wA[g][:, j * 256:(j + 1) * 256],
                        start=(j == 0), stop=(j == 3),
                    )
                sA = wp.tile([128, 256], F32, tag="sbA", name=f"sbA{g}", bufs=4)
                nc.vector.tensor_copy(out=sA[:P, :], in_=pA[:P, :])
                sbA.append(sA)

            # transpose back to [H, n] layout: psum1[ob][:, g] = sbA_g.T
            psum1 = [ps1.tile([128, N], F32, tag=f"p1{ob}", name=f"p1{ob}") for ob in range(2)]
            for g in range(MPC):
                for ob in range(2):
                    nc.tensor.transpose(
                        psum1[ob][:, gsl(g)], sbA[g][:P, ob * 128:(ob + 1) * 128],
                        xin[:P, c_id:c_id + P],
                    )

            # phase A epilogue: hid = tanh(psum + enc + (bc+br)); bf16 copy for phase B
            hid, hidb = [], []
            for ob in range(2):
                pre = wp.tile([128, N], F32, tag="pre", name="pre")
                nc.vector.tensor_add(
                    out=pre[:, :], in0=psum1[ob][:, :],
                    in1=xin[:, c_enc + ob * N:c_enc + (ob + 1) * N],
                )
                for g in range(MPC):
                    nc.vector.tensor_scalar_add(
                        pre[:, gsl(g)], pre[:, gsl(g)],
                        xin[:, c_bh + ob * 4 + g:c_bh + ob * 4 + g + 1],
                    )
                h = pp.tile([128, N], F32, tag=f"hid{ob}", name=f"hid{ob}")
                nc.scalar.activation(h[:, :], pre[:, :], mybir.ActivationFunctionType.Tanh)
                nc.sync.dma_start(out=hid_d[ob, :, :], in_=h[:, :])
                hb = pp.tile([128, N], BF16, tag=f"hidb{ob}", name=f"hidb{ob}")
                nc.vector.tensor_copy(out=hb[:, :], in_=h[:, :])
                hid.append(h)
                hidb.append(hb)

            # action|baseline: [17, N] = [Wa|Wb].T @ hid + [ba; bb]
            pab = ps1.tile([AB, N], F32, tag="pab", name="pab")
            for g in range(MPC):
                for i in range(2):
                    wt = wab[:, (g * 2 + i) * AB:(g * 2 + i + 1) * AB]
                    nc.tensor.matmul(
                        pab[:, gsl(g)], wt, hidb[i][:, gsl(g)],
                        start=(i == 0), stop=(i == 1),
                    )
            absb = pp.tile([AB, N], F32, tag="absb", name="absb")
            for g in range(MPC):
                nc.vector.tensor_scalar_add(
                    absb[:, gsl(g)], pab[:, gsl(g)], xin[:AB, c_ab + g:c_ab + g + 1]
                )
            # transpose chunks of <=128 examples; softmax rows (free dim = 16)
            for ch0 in range(0, N, 128):
                cw = min(128, N - ch0)
                ptr = ptrp.tile([128, AB], F32, tag="ptr", name="ptr")
                nc.tensor.transpose(
                    ptr[:cw, :], absb[:, ch0:ch0 + cw], xin[:AB, c_id:c_id + AB]
                )
                ex = wp.tile([128, NACT], F32, tag="ex", name="ex")
                nc.scalar.activation(
                    ex[:cw, :], ptr[:cw, :NACT], mybir.ActivationFunctionType.Exp
                )
                red = wp.tile([128, 1], F32, tag="red", name="red")
                nc.vector.reduce_sum(out=red[:cw, :], in_=ex[:cw, :], axis=mybir.AxisListType.X)
                rec = wp.tile([128, 1], F32, tag="rec", name="rec")
                nc.vector.reciprocal(rec[:cw, :], red[:cw, :])
                aout = wp.tile([128, AB], F32, tag="aout", name="aout")
                nc.vector.tensor_scalar_mul(aout[:cw, :NACT], ex[:cw, :], rec[:cw, :])
                nc.vector.tensor_copy(out=aout[:cw, NACT:AB], in_=ptr[:cw, NACT:AB])
                nc.sync.dma_start(out=act_d[ch0:ch0 + cw, :], in_=aout[:cw, :])

            # comm_out: ob-major streaming; co = (psum + bo) / (NAG-1)
            for ob in range(16):
                po = pop.tile([128, N], F32, tag="po", name="po")
                for g in range(MPC):
                    for i in range(2):
                        wt = wO[ob][:, (g * 2 + i) * 128:(g * 2 + i + 1) * 128]
                        nc.tensor.matmul(
                            po[:, gsl(g)], wt, hidb[i][:, gsl(g)],
                            start=(i == 0), stop=(i == 1),
                        )
                co = cop.tile([128, N], F32, tag="co", name="co")
                for g in range(MPC):
                    nc.vector.tensor_scalar(
                        co[:, gsl(g)], po[:, gsl(g)],
                        xin[:, c_bo + g * 16 + ob:c_bo + g * 16 + ob + 1],
                        1.0 / (NAG - 1),
                        op0=mybir.AluOpType.add, op1=mybir.AluOpType.mult,
                    )
                nc.sync.dma_start(out=com_d[ob, :, :], in_=co[:, :])

    _split_sync_waits(nc)
    layout = dict(N=N, c_comm=c_comm, c_hidp=c_hidp, c_enc=c_enc, c_bh=c_bh,
                  c_bo=c_bo, c_id=c_id, c_ab=c_ab, XCOLS=XCOLS,
                  c_wo=c_wo, CWB=CWB)
    return nc, layout


def _pack_core(L, P, models, groups, comm_in, prev_hid, encg,
               Wc, bc, Wr, br, Wa, ba, Wb, bb, Wo, bo):
    """Build (xin, wbig) for one core."""
    N = L["N"]
    sel = np.zeros(N, np.int64)
    nvalid = []
    for g, m in enumerate(models):
        ii = groups[m]
        sel[g * P:g * P + len(ii)] = ii
        nvalid.append(len(ii))
    valid = np.zeros(N, bool)
    for g in range(MPC):
        valid[g * P:g * P + nvalid[g]] = True

    xin = np.zeros((128, L["XCOLS"]), np.float32)
    cg = comm_in[sel]          # [N, 8, 256]
    cg[~valid] = 0
    # [N,8,256] -> [128, (a,i,n)]
    xin[:, L["c_comm"]:L["c_comm"] + 16 * N] = (
        cg.reshape(N, NAG, 2, 128).transpose(3, 1, 2, 0).reshape(128, 16 * N)
    )
    hg = prev_hid[sel]
    hg[~valid] = 0
    xin[:, L["c_hidp"]:L["c_hidp"] + 2 * N] = (
        hg.reshape(N, 2, 128).transpose(2, 1, 0).reshape(128, 2 * N)
    )
    eg = encg[sel]
    eg[~valid] = 0
    xin[:, L["c_enc"]:L["c_enc"] + 2 * N] = (
        eg.reshape(N, 2, 128).transpose(2, 1, 0).reshape(128, 2 * N)
    )
    bh = bc[models] + br[models]  # [4, 256]
    xin[:, L["c_bh"]:L["c_bh"] + 8] = bh.reshape(MPC, 2, 128).transpose(2, 1, 0).reshape(128, 8)
    xin[:, L["c_bo"]:L["c_bo"] + 64] = (
        bo[models].reshape(MPC, 16, 128).transpose(2, 0, 1).reshape(128, 64)
    )
    xin[:, L["c_id"]:L["c_id"] + 128] = np.eye(128, dtype=np.float32)
    xin[:NACT, L["c_ab"]:L["c_ab"] + MPC] = ba[models].T
    xin[NACT, L["c_ab"]:L["c_ab"] + MPC] = bb[models, 0]

    import ml_dtypes
    # phase A (flipped): per g, j-blocks [128, 256]: Wc i0, Wc i1, Wr i0, Wr i1
    wa = np.empty((128, MPC * 1024), np.float32)
    for g, m in enumerate(models):
        for j, (W, i) in enumerate([(Wc, 0), (Wc, 1), (Wr, 0), (Wr, 1)]):
            wa[:, g * 1024 + j * 256:g * 1024 + (j + 1) * 256] = (
                W[m, i * 128:(i + 1) * 128, :]
            )
    # phase B (bf16): [wab | Wo ob-major]
    wb = np.empty((128, L["CWB"]), ml_dtypes.bfloat16)
    for g, m in enumerate(models):
        for i in range(2):
            c0 = (g * 2 + i) * AB
            wb[:, c0:c0 + NACT] = Wa[m, i * 128:(i + 1) * 128, :]
            wb[:, c0 + NACT:c0 + AB] = Wb[m, i * 128:(i + 1) * 128, :]
    for ob in range(16):
        c0 = L["c_wo"] + ob * 1024
        for g, m in enumerate(models):
            for i in range(2):
                wb[:, c0 + (g * 2 + i) * 128:c0 + (g * 2 + i + 1) * 128] = (
                    Wo[m, i * 128:(i + 1) * 128, ob * 128:(ob + 1) * 128]
                )
    return xin, wa, wb


def kernel(comm_in, inp, prev_hid, prev_cell, model_ids,
           Wc, bc, Wr, br, Wa, ba, Wb, bb, Wo, bo, lut, enc_bias):
    global LAST_RESULTS
    f = lambda x: np.ascontiguousarray(np.asarray(x), dtype=np.float32)
    comm_in, prev_hid = f(comm_in), f(prev_hid)
    Wc, bc, Wr, br = f(Wc), f(bc), f(Wr), f(br)
    Wa, ba, Wb, bb, Wo, bo = f(Wa), f(ba), f(Wb), f(bb), f(Wo), f(bo)
    lut, enc_bias = f(lut), f(enc_bias)
    ids = np.asarray(model_ids).astype(np.int64)
    inp_i = np.asarray(inp).astype(np.int64)

    # Encoder (embedding gather + bias) — index logistics done host-side.
    encg = lut[np.clip(inp_i[:, 0], 0, lut.shape[0] - 1)] + enc_bias  # [B, 256]

    # Greedy balanced assignment of models to cores.
    counts = np.bincount(ids, minlength=M)
    groups = [np.where(ids == m)[0] for m in range(M)]
    order = np.argsort(-counts, kind="stable")
    loads, core_models = [0] * NCORES, [[] for _ in range(NCORES)]
    for m in order:
        cands = [c for c in range(NCORES) if len(core_models[c]) < MPC]
        c = min(cands, key=lambda c: loads[c])
        core_models[c].append(int(m))
        loads[c] += int(counts[m])
    P = max(4, int(-(-int(counts.max()) // 4)) * 4)

    key = P
    if key not in _PROG_CACHE:
        _PROG_CACHE[key] = _build_program(P)
    nc, L = _PROG_CACHE[key]
    N = L["N"]

    in_maps = []
    for c in range(NCORES):
        xin, wa, wb = _pack_core(L, P, core_models[c], groups, comm_in, prev_hid,
                                 encg, Wc, bc, Wr, br, Wa, ba, Wb, bb, Wo, bo)
        in_maps.append({"xin": xin, "wa": wa, "wb": wb})

    res = run_bass_kernel_spmd(nc, in_maps, core_ids=list(range(NCORES)), trace=TRACE)
    LAST_RESULTS = res

    action = np.zeros((B, NACT), np.float32)
    baseline = np.zeros((B, 1), np.float32)
    hidstate = np.zeros((B, H), np.float32)
    comm_out = np.zeros((B, O), np.float32)
    for c in range(NCORES):
        oact = res.results[c]["oact"]            # [N, 17]
        ohid = res.results[c]["ohid"].reshape(H, N)
        ocom = res.results[c]["ocom"].reshape(O, N)
        for g, m in enumerate(core_models[c]):
            ii = groups[m]
            n = len(ii)
            if n == 0:
                continue
            r = slice(g * P, g * P + n)
            action[ii] = oact[r, :NACT]
            baseline[ii, 0] = oact[r, NACT]
            hidstate[ii] = ohid[:, r].T
            comm_out[ii] = ocom[:, r].T
    return action, baseline, hidstate, comm_out
